# revision 22
# baseline (speedup 1.0000x reference)
"""Trainium2 Bass kernel for BoxMultiHeadedAttention (B=4, S=1024, D=1024, H=16).

Reference math (eval mode, mask is all-ones so the masking is a no-op):
    qg/kg/qa/ka/va = per-head projections of the five inputs
    q = concat([qa, qg], -1); k = concat([ka, kg], -1)           # [B,H,S,128]
    p = softmax(q @ k.T / sqrt(128)); x = (p @ va) -> [B,S,D]
    out = sigmoid(concat([query_a, query_g], -1) @ Wgate.T + bgate) * (x @ Winfo.T + binfo)

Sharding: 8 cores = 4 batches x 2 head-halves. Core c handles batch c//2 and
heads (c%2)*8 .. +8 (which are also x-columns (c%2)*512..+512).  The GLU is
column-sharded the same way; the attention output is exchanged between core
pairs with per-head pairwise AllGathers so each core can compute its 512
output columns of fc_info (which contracts over all 1024 x-dims).

Layout: everything is computed transposed ([feature, seq] with feature on
partitions).  Scores are computed k-major (sT = k @ q.T), the softmax
denominator comes from a ones-column appended to va (row 64 of the p@v
accumulation), and the normalization happens after the p@v matmul on the
small [64, S] output.  bva folds into an adjusted fc_info bias on the host.

Schedule: q/k projections are issued per-head and interleaved with that
head's attention, so the ACT exp stream starts as soon as head 0's q/k are
ready and overlaps the remaining projections on the PE.  The softmax
denominator reciprocal runs on the DVE (reciprocal_approx_fast), keeping the
ACT table set pinned to exp for the whole attention phase; the gate matmuls
stash raw scores and all sigmoids run in one batch at the tail (a single
table switch) while the final AllGather is in flight.
"""

import os

import ml_dtypes
import numpy as np

import concourse.bass as bass
import concourse.mybir as mybir
import concourse.tile as tile
from concourse.tile import add_dep_helper
from concourse import bacc, bass_utils

B, S, D, H = 4, 1024, 1024, 16
DK = D // H            # 64
CD = 2 * DK            # 128 concat head dim
HL = H // 2            # 8 local heads per core
T = D // 128           # 8 partition tiles per 1024 dim
NQ = S // 512          # 2 moving-dim blocks
SCALE = 1.0 / float(np.sqrt(2 * DK))

F32 = mybir.dt.float32
F32R = mybir.dt.float32r
BF16 = mybir.dt.bfloat16
NPBF16 = ml_dtypes.bfloat16

REPLICA_GROUPS = [[0, 1], [2, 3], [4, 5], [6, 7]]


def build_nc():
    nc = bacc.Bacc("TRN2", target_bir_lowering=False, debug=False, num_devices=8)

    # ---- DRAM I/O (per-core tensors; same program on all 8 cores) ----
    # big operands are laid out partition-major on the host ([128, T*n]) so
    # each DMA moves long contiguous per-partition lines at full HBM rate
    d_xqa = nc.dram_tensor("xqa", [128, T * S], BF16, kind="ExternalInput")
    d_xqg = nc.dram_tensor("xqg", [128, T * S], BF16, kind="ExternalInput")
    d_xka = nc.dram_tensor("xka", [128, T * S], BF16, kind="ExternalInput")
    d_xkg = nc.dram_tensor("xkg", [128, T * S], BF16, kind="ExternalInput")
    d_xv = nc.dram_tensor("xv", [128, T * S], BF16, kind="ExternalInput")
    d_wqa = nc.dram_tensor("wqa", [128, T * 512], BF16, kind="ExternalInput")
    d_wqg = nc.dram_tensor("wqg", [128, T * 512], BF16, kind="ExternalInput")
    d_wka = nc.dram_tensor("wka", [128, T * 512], BF16, kind="ExternalInput")
    d_wkg = nc.dram_tensor("wkg", [128, T * 512], BF16, kind="ExternalInput")
    d_wv = nc.dram_tensor("wv", [128, T * 512], BF16, kind="ExternalInput")
    d_wg = nc.dram_tensor("wg", [128, 2 * T * 512], BF16, kind="ExternalInput")
    d_wi = nc.dram_tensor("wi", [128, T * 512], BF16, kind="ExternalInput")
    d_bq = nc.dram_tensor("bq", [CD, HL], F32, kind="ExternalInput")
    d_bk = nc.dram_tensor("bk", [CD, HL], F32, kind="ExternalInput")
    d_bg = nc.dram_tensor("bg", [128, 4], F32, kind="ExternalInput")
    d_bi = nc.dram_tensor("bi", [128, 4], F32, kind="ExternalInput")
    d_out = nc.dram_tensor("out", [4, 128, S], F32, kind="ExternalOutput")

    with tile.TileContext(nc) as tc:
        with (
            tc.tile_pool(name="xin", bufs=1) as p_xin,
            tc.tile_pool(name="wts", bufs=1) as p_w,
            tc.tile_pool(name="big", bufs=1) as p_big,
            tc.tile_pool(name="att", bufs=1) as p_att,
            tc.tile_pool(name="tail", bufs=1) as p_tail,
            tc.tile_pool(name="psA", bufs=1, space="PSUM") as p_psA,
            tc.tile_pool(name="psS", bufs=1, space="PSUM") as p_psS,
            tc.tile_pool(name="psX", bufs=1, space="PSUM") as p_psX,
            tc.tile_pool(name="dram", bufs=1, space="DRAM") as p_dram,
        ):
            # --- persistent sbuf tiles (tags control slot reuse) ---
            t_xv = p_xin.tile([128, T, S], BF16, tag="vin", bufs=1)
            t_xqa = p_xin.tile([128, T, S], BF16, tag="qin", bufs=2)
            t_xqg = p_xin.tile([128, T, S], BF16, tag="qin", bufs=2)
            t_xka = p_xin.tile([128, T, S], BF16, tag="kin", bufs=2)
            t_xkg = p_xin.tile([128, T, S], BF16, tag="kin", bufs=2)

            t_wv = p_w.tile([128, T, 512], BF16, tag="w8", bufs=5)
            t_wqa = p_w.tile([128, T, 512], BF16, tag="w8", bufs=5)
            t_wqg = p_w.tile([128, T, 512], BF16, tag="w8", bufs=5)
            t_wka = p_w.tile([128, T, 512], BF16, tag="w8", bufs=5)
            t_wkg = p_w.tile([128, T, 512], BF16, tag="w8", bufs=5)
            t_wi = p_w.tile([128, T, 512], BF16, tag="w8", bufs=5)

            t_bq = p_w.tile([CD, HL], F32, tag="bias", bufs=4)
            t_bk = p_w.tile([CD, HL], F32, tag="bias", bufs=4)
            t_bg = p_w.tile([128, 4], F32, tag="bias", bufs=4)
            t_bi = p_w.tile([128, 4], F32, tag="bias", bufs=4)

            t_va = p_att.tile([128, T, HL, DK + 1], BF16, tag="va", bufs=1)
            # gathered x (all 8 x-dim blocks; the local half is written by
            # the normalize mult, then the exchange re-writes it (same
            # bytes) and fills the partner half)
            t_xtf = p_big.tile([128, T, S], BF16, tag="xtf", bufs=1)
            t_graw = p_w.tile([128, 4, S], BF16, tag="w8", bufs=5)

            # --- PE warmup: keep TensorE busy during the DMA lead-in so HAM
            # un-throttles before the real matmuls arrive ---
            t_wu = p_att.tile([128, 256], BF16, tag="wu", bufs=1)
            nc.vector.memset(t_wu[:], 0.0)
            for _ in range(24):
                pwu = p_psA.tile([128, 256], F32, tag="proj", bufs=2)
                nc.tensor.matmul(pwu[:], t_wu[:, 0:128], t_wu[:],
                                 start=True, stop=True)

            # --- load inputs / weights, in consumption order; 1MB chunks
            # so several DMA queues run in parallel ---
            def load(dt_, tl, n_t, chunk=4):
                r = dt_.ap().rearrange("p (t n) -> p t n", t=n_t)
                for tt in range(0, n_t, chunk):
                    nc.sync.dma_start(tl[:, tt:tt + chunk, :],
                                      r[:, tt:tt + chunk, :])

            def load2(da, ta, db, tb, n_t, chunk=2):
                ra = da.ap().rearrange("p (t n) -> p t n", t=n_t)
                rb = db.ap().rearrange("p (t n) -> p t n", t=n_t)
                for tt in range(0, n_t, chunk):
                    nc.sync.dma_start(ta[:, tt:tt + chunk, :],
                                      ra[:, tt:tt + chunk, :])
                    nc.sync.dma_start(tb[:, tt:tt + chunk, :],
                                      rb[:, tt:tt + chunk, :])

            def load_whead(h):
                for dt_, tl in ((d_wqa, t_wqa), (d_wqg, t_wqg),
                                (d_wka, t_wka), (d_wkg, t_wkg)):
                    r = dt_.ap().rearrange("p (t n) -> p t n", t=T)
                    nc.sync.dma_start(tl[:, :, h * DK:(h + 1) * DK],
                                      r[:, :, h * DK:(h + 1) * DK])

            load(d_wv, t_wv, T)
            load(d_xv, t_xv, T)
            for h in range(3):
                load_whead(h)
            load2(d_xqa, t_xqa, d_xqg, t_xqg, T)
            load2(d_xka, t_xka, d_xkg, t_xkg, T)
            for h in range(3, HL):
                load_whead(h)
            t_wg = p_xin.tile([128, 2 * T, 512], BF16, tag="vin", bufs=1)
            load(d_wg, t_wg, 2 * T)
            load(d_wi, t_wi, T)
            nc.sync.dma_start(t_bq[:], d_bq.ap())
            nc.sync.dma_start(t_bk[:], d_bk.ap())
            nc.sync.dma_start(t_bg[:], d_bg.ap())
            nc.sync.dma_start(t_bi[:], d_bi.ap())

            # --- q/k projections (transposed, concat layout) ---
            # psum rows 0:64 <- a-head dims (weights col-tile 0), rows 64:128
            # <- g-head dims (col-tile 64); the two M=64 matmuls per step run
            # concurrently in distinct PE column groups.
            def proj_head(h, wa, wb, xa, xb, bias, name):
                nbufs = 8 if name == "qT" else 3
                dst = p_big.tile([128, S], BF16, tag=name, bufs=nbufs,
                                 name=f"{name}_{h}")
                for n in range(NQ):
                    ps = p_psA.tile([128, 512], F32, tag="proj", bufs=2)
                    for kt in range(T):
                        nc.tensor.matmul(
                            ps[0:64, :],
                            wa[:, kt, h * DK:(h + 1) * DK],
                            xa[:, kt, n * 512:(n + 1) * 512],
                            start=(kt == 0), stop=(kt == T - 1),
                            tile_position=(0, 0), skip_group_check=True,
                        )
                        nc.tensor.matmul(
                            ps[64:128, :],
                            wb[:, kt, h * DK:(h + 1) * DK],
                            xb[:, kt, n * 512:(n + 1) * 512],
                            start=(kt == 0), stop=(kt == T - 1),
                            tile_position=(0, 64), skip_group_check=True,
                        )
                    nc.vector.tensor_scalar_add(
                        dst[:, n * 512:(n + 1) * 512], ps[:],
                        bias[:, h:h + 1],
                    )
                return dst

            # --- va projection (natural [s, dk] layout, + ones column) ---
            nc.vector.memset(t_va[:, :, :, DK:DK + 1], 1.0)
            for st in range(T):
                ps = p_psA.tile([128, 512], F32, tag="proj", bufs=2)
                for kt in range(T):
                    nc.tensor.matmul(
                        ps[:],
                        t_xv[:, kt, st * 128:(st + 1) * 128],
                        t_wv[:, kt, :],
                        start=(kt == 0), stop=(kt == T - 1),
                    )
                nc.vector.tensor_copy(
                    t_va[:, st, :, 0:DK],
                    ps[:].rearrange("p (h d) -> p h d", h=HL),
                )

            # fillers pinned behind va bridge the xka/xkg DMA window
            for _ in range(12):
                pwu = p_psA.tile([128, 256], F32, tag="proj", bufs=2,
                                 name="pwu_fill")
                nc.tensor.matmul(pwu[0:DK + 1, :], t_va[:, 0, 0, :], t_wu[:],
                                 start=True, stop=True)

            # q for all heads first: depends only on the earliest DMAs, so
            # the PE has dense work while the k/v inputs stream in
            all_q = [proj_head(h, t_wqa, t_wqg, t_xqa, t_xqg, t_bq, "qT")
                     for h in range(HL)]

            # fillers pinned behind the q block bridge the xv DMA window
            for _ in range(8):
                pwu = p_psA.tile([128, 256], F32, tag="proj", bufs=2,
                                 name="pwu_fq")
                nc.tensor.matmul(pwu[:], all_q[7][:, 0:128], t_wu[:],
                                 start=True, stop=True)

            # --- attention (k-major scores, flash-style over k tiles) ---
            def attn_head(h, t_qh, t_kh):
                px = p_psX.tile([DK + 1, S], F32, tag="x", bufs=2,
                                name=f"px_{h}")
                for kt in range(T):
                    te = p_att.tile([128, S], BF16, tag="exp", bufs=3,
                                    name=f"te_{h}_{kt}")
                    for n in range(NQ):
                        pss = p_psS.tile([128, 512], F32, tag="s", bufs=2,
                                         name=f"pss_{h}_{kt}_{n}")
                        nc.tensor.matmul(
                            pss[:],
                            t_kh[:, kt * 128:(kt + 1) * 128],
                            t_qh[:, n * 512:(n + 1) * 512],
                            start=True, stop=True,
                        )
                        nc.scalar.activation(
                            te[:, n * 512:(n + 1) * 512], pss[:],
                            mybir.ActivationFunctionType.Exp, scale=SCALE,
                        )
                    for n in range(NQ):
                        nc.tensor.matmul(
                            px[:, n * 512:(n + 1) * 512],
                            t_va[:, kt, h, :],
                            te[:, n * 512:(n + 1) * 512],
                            start=(kt == 0), stop=(kt == T - 1),
                        )
                # normalize: row DK of px holds the softmax denominator.
                # The reciprocal runs on the (otherwise idle) DVE, keeping
                # the ACT table set pinned to exp all through attention.
                # (The custom DVE op misreads PSUM at non-zero bank offsets,
                # so the row is staged through SBUF first.)
                t_den = p_att.tile([1, S], F32, tag="den", bufs=1,
                                   name=f"den_{h}")
                nc.vector.tensor_copy(t_den[:], px[DK:DK + 1, :])
                t_rcp = p_att.tile([1, S], F32, tag="recip", bufs=1,
                                   name=f"recip_{h}")
                nc.vector.reciprocal_approx_fast(t_rcp[:], t_den[:])
                for n in range(NQ):
                    t_bc = p_att.tile([DK, 512], F32, tag="bc", bufs=1,
                                      name=f"bc_{h}_{n}")
                    nc.gpsimd.partition_broadcast(
                        t_bc[:], t_rcp[0:1, n * 512:(n + 1) * 512])
                    mult_inst = nc.vector.tensor_tensor(
                        t_xtf[(h % 2) * DK:(h % 2) * DK + DK, h // 2,
                              n * 512:(n + 1) * 512],
                        px[0:DK, n * 512:(n + 1) * 512], t_bc[:],
                        op=mybir.AluOpType.mult,
                    )
                return mult_inst

            # Ship each head's x block with its own pairwise AllGather as
            # soon as it completes: early heads hide entirely under the rest
            # of attention; the last head's exchange is covered by the gate
            # sigmoids and the info tail below.
            def ship_head(h):
                lo = (h % 2) * DK
                cc_in = p_dram.tile([1, DK, S], BF16, name=f"cci_{h}")
                cc_out = p_dram.tile([2, DK, S], BF16, name=f"cco_{h}")
                nc.sync.dma_start(cc_in[0], t_xtf[lo:lo + DK, h // 2, :])
                nc.gpsimd.collective_compute(
                    "AllGather", mybir.AluOpType.bypass,
                    replica_groups=REPLICA_GROUPS,
                    ins=[cc_in[:].opt()], outs=[cc_out[:].opt()],
                )
                nc.sync.dma_start(t_xtf[lo:lo + DK, h // 2, :], cc_out[0])
                nc.sync.dma_start(t_xtf[lo:lo + DK, 4 + h // 2, :], cc_out[1])

            head_done = []
            for h in range(HL):
                t_kh = proj_head(h, t_wka, t_wkg, t_xka, t_xkg, t_bk, "kT")
                head_done.append(attn_head(h, all_q[h], t_kh))
                ship_head(h)

            # --- gate matmuls (fill PE slack in the ACT-paced attention);
            # raw scores stashed, sigmoids batched at the tail so the ACT
            # pays a single table switch, inside the last-exchange window ---
            def gate_group(mt, n, anchor):
                ps = p_psA.tile([128, 512], F32, tag="proj", bufs=2,
                                name=f"gate_{mt}_{n}")
                for kt in range(2 * T):
                    xsrc = t_xqa if kt < T else t_xqg
                    mm = nc.tensor.matmul(
                        ps[:],
                        t_wg[:, kt, mt * 128:(mt + 1) * 128],
                        xsrc[:, kt % T, n * 512:(n + 1) * 512],
                        start=(kt == 0), stop=(kt == 2 * T - 1),
                    )
                    if kt == 0 and anchor is not None:
                        add_dep_helper(mm.ins, anchor.ins, sync=True,
                                       reason="gate after mid-attention")
                nc.vector.tensor_copy(
                    t_graw[:, mt, n * 512:(n + 1) * 512], ps[:])

            # scheduling-only anchors: gate fills the stalls of the LAST few
            # attention heads and the exchange window, never the early heads
            # (which set the pace of the serializing per-head AllGathers)
            gate_anchor = [2, 3, 4, 5, 5, 6, 6, 6]
            for i, (mt, n) in enumerate((m, nn) for m in range(4)
                                        for nn in range(NQ)):
                gate_group(mt, n, head_done[gate_anchor[i]])

            t_gate = p_xin.tile([128, 4, S], BF16, tag="kin", bufs=2)
            for mt in range(4):
                nc.scalar.activation(
                    t_gate[:, mt, :], t_graw[:, mt, :],
                    mybir.ActivationFunctionType.Sigmoid,
                    bias=t_bg[:, mt:mt + 1],
                )

            # --- info, split so only x-blocks 3/7 are contracted after the
            # last AllGather lands; blocks {0,4,1,5,2,6} pre-accumulate into
            # SBUF during late attention ---
            t_acc = p_tail.tile([128, 4, NQ, 512], BF16, tag="acc", bufs=1)
            for mt in range(4):
                for n in range(NQ):
                    ps = p_psA.tile([128, 512], F32, tag="proj", bufs=2,
                                    name=f"infoA_{mt}_{n}")
                    for i, kt in enumerate((0, 4, 1, 5, 2, 6)):
                        mm = nc.tensor.matmul(
                            ps[:],
                            t_wi[:, kt, mt * 128:(mt + 1) * 128],
                            t_xtf[:, kt, n * 512:(n + 1) * 512],
                            start=(i == 0), stop=(i == 5),
                        )
                        if i == 0:
                            add_dep_helper(mm.ins, head_done[5].ins, sync=True,
                                           reason="infoA after head 5")
                    nc.vector.tensor_copy(t_acc[:, mt, n, :], ps[:])
            for mt in range(4):
                for n in range(NQ):
                    ps = p_psA.tile([128, 512], F32, tag="proj", bufs=2,
                                    name=f"infoB_{mt}_{n}")
                    for i, kt in enumerate((3, 7)):
                        nc.tensor.matmul(
                            ps[:],
                            t_wi[:, kt, mt * 128:(mt + 1) * 128],
                            t_xtf[:, kt, n * 512:(n + 1) * 512],
                            start=(i == 0), stop=(i == 1),
                        )
                    t_sb = p_tail.tile([128, 512], F32, tag="osum", bufs=2)
                    nc.vector.scalar_tensor_tensor(
                        t_sb[:], ps[:], t_bi[:, mt:mt + 1],
                        t_acc[:, mt, n, :],
                        op0=mybir.AluOpType.add, op1=mybir.AluOpType.add,
                    )
                    t_ob = p_tail.tile([128, 512], F32, tag="outb", bufs=2)
                    nc.vector.tensor_tensor(
                        t_ob[:], t_sb[:],
                        t_gate[:, mt, n * 512:(n + 1) * 512],
                        op=mybir.AluOpType.mult,
                    )
                    nc.sync.dma_start(
                        d_out.ap()[mt, :, n * 512:(n + 1) * 512], t_ob[:])

    nc.compile()
    return nc


def make_in_maps(inputs):
    """Host-side sharding: transpose/slice/cast the full inputs per core."""
    f32 = np.float32
    g = {k: np.asarray(v) for k, v in inputs.items()}
    binfo_eff = (
        g["binfo"].astype(np.float64)
        + g["Winfo"].astype(np.float64) @ g["bva"].astype(np.float64)
    ).astype(f32)

    in_maps = []
    for c in range(8):
        b, hh = c // 2, c % 2
        hs = slice(hh * 512, (hh + 1) * 512)

        def pmajor(a):
            # [1024*k, n] -> partition-major [128, k*T*n]-style layout the
            # kernel DMAs as long contiguous per-partition lines
            rows, n = a.shape
            t = rows // 128
            return np.ascontiguousarray(
                a.reshape(t, 128, n).transpose(1, 0, 2).reshape(128, t * n))

        def xt(name):
            return pmajor(g[name][b].T.astype(NPBF16))

        def wt(name):
            return pmajor(g[name][hs].T.astype(NPBF16))

        def bqk(pa, pg):
            a = g[pa][hs].reshape(HL, DK).T.astype(f32)   # [64, 8]
            gg = g[pg][hs].reshape(HL, DK).T.astype(f32)
            return np.ascontiguousarray(np.vstack([a, gg]))  # [128, 8]

        m = {
            "xqa": xt("query_a"), "xqg": xt("query_g"),
            "xka": xt("key_a"), "xkg": xt("key_g"), "xv": xt("value_a"),
            "wqa": wt("Wqa"), "wqg": wt("Wqg"),
            "wka": wt("Wka"), "wkg": wt("Wkg"), "wv": wt("Wva"),
            "wg": wt("Wgate"), "wi": wt("Winfo"),
            "bq": bqk("bqa", "bqg"), "bk": bqk("bka", "bkg"),
            "bg": np.ascontiguousarray(
                g["bgate"][hs].reshape(4, 128).T.astype(f32)),
            "bi": np.ascontiguousarray(
                binfo_eff[hs].reshape(4, 128).T.astype(f32)),
        }
        in_maps.append(m)
    return in_maps


def assemble(results):
    out = np.empty((B, S, D), dtype=np.float32)
    for c in range(8):
        b, hh = c // 2, c % 2
        blk = results[c]["out"].reshape(512, S)   # [cols, seq]
        out[b, :, hh * 512:(hh + 1) * 512] = blk.T
    return out


_NC_CACHE = {}


def _get_nc():
    if "nc" not in _NC_CACHE:
        _NC_CACHE["nc"] = build_nc()
    return _NC_CACHE["nc"]


LAST_RESULTS = None


def kernel(**inputs) -> np.ndarray:
    global LAST_RESULTS
    nc = _get_nc()
    in_maps = make_in_maps(inputs)
    trace = os.environ.get("KERNEL_TRACE", "0") == "1"
    kwargs = {}
    if trace:
        kwargs["trace_cores"] = list(range(8))
    res = bass_utils.run_bass_kernel_spmd(
        nc, in_maps, core_ids=list(range(8)), trace=trace, **kwargs,
    )
    LAST_RESULTS = res
    return assemble(res.results)


# revision 23
# speedup vs baseline: 1.2297x; 1.2297x over previous
"""Trainium2 Bass kernel for BoxMultiHeadedAttention (B=4, S=1024, D=1024, H=16).

Reference math (eval mode, mask is all-ones so the masking is a no-op):
    qg/kg/qa/ka/va = per-head projections of the five inputs
    q = concat([qa, qg], -1); k = concat([ka, kg], -1)           # [B,H,S,128]
    p = softmax(q @ k.T / sqrt(128)); x = (p @ va) -> [B,S,D]
    out = sigmoid(concat([query_a, query_g], -1) @ Wgate.T + bgate) * (x @ Winfo.T + binfo)

Sharding: 8 cores = 4 batches x 2 head-halves. Core c handles batch c//2 and
heads (c%2)*8 .. +8 (which are also x-columns (c%2)*512..+512).  The GLU is
column-sharded the same way; the attention output is exchanged between core
pairs with per-head pairwise AllGathers so each core can compute its 512
output columns of fc_info (which contracts over all 1024 x-dims).

Layout: everything is computed transposed ([feature, seq] with feature on
partitions).  Scores are computed k-major (sT = k @ q.T), the softmax
denominator comes from a ones-column appended to va (row 64 of the p@v
accumulation), and the normalization happens after the p@v matmul on the
small [64, S] output.  bva folds into an adjusted fc_info bias on the host.

Schedule: q/k projections are issued per-head and interleaved with that
head's attention, so the ACT exp stream starts as soon as head 0's q/k are
ready and overlaps the remaining projections on the PE.  The softmax
denominator reciprocal runs on the DVE (reciprocal_approx_fast), keeping the
ACT table set pinned to exp for the whole attention phase; the gate matmuls
stash raw scores and all sigmoids run in one batch at the tail (a single
table switch) while the final AllGather is in flight.
"""

import os

import ml_dtypes
import numpy as np

import concourse.bass as bass
import concourse.mybir as mybir
import concourse.tile as tile
from concourse.tile import add_dep_helper
from concourse import bacc, bass_utils

B, S, D, H = 4, 1024, 1024, 16
DK = D // H            # 64
CD = 2 * DK            # 128 concat head dim
HL = H // 2            # 8 local heads per core
T = D // 128           # 8 partition tiles per 1024 dim
NQ = S // 512          # 2 moving-dim blocks
SCALE = 1.0 / float(np.sqrt(2 * DK))

F32 = mybir.dt.float32
F32R = mybir.dt.float32r
BF16 = mybir.dt.bfloat16
NPBF16 = ml_dtypes.bfloat16

REPLICA_GROUPS = [[0, 1], [2, 3], [4, 5], [6, 7]]


def build_nc():
    nc = bacc.Bacc("TRN2", target_bir_lowering=False, debug=False, num_devices=8)

    # ---- DRAM I/O (per-core tensors; same program on all 8 cores) ----
    # big operands are laid out partition-major on the host ([128, T*n]) so
    # each DMA moves long contiguous per-partition lines at full HBM rate
    d_xqa = nc.dram_tensor("xqa", [128, T * S], BF16, kind="ExternalInput")
    d_xqg = nc.dram_tensor("xqg", [128, T * S], BF16, kind="ExternalInput")
    d_xka = nc.dram_tensor("xka", [128, T * S], BF16, kind="ExternalInput")
    d_xkg = nc.dram_tensor("xkg", [128, T * S], BF16, kind="ExternalInput")
    d_xv = nc.dram_tensor("xv", [128, T * S], BF16, kind="ExternalInput")
    d_wqa = nc.dram_tensor("wqa", [128, T * 512], BF16, kind="ExternalInput")
    d_wqg = nc.dram_tensor("wqg", [128, T * 512], BF16, kind="ExternalInput")
    d_wka = nc.dram_tensor("wka", [128, T * 512], BF16, kind="ExternalInput")
    d_wkg = nc.dram_tensor("wkg", [128, T * 512], BF16, kind="ExternalInput")
    d_wv = nc.dram_tensor("wv", [128, T * 512], BF16, kind="ExternalInput")
    d_wg = nc.dram_tensor("wg", [128, 2 * T * 512], BF16, kind="ExternalInput")
    d_wi = nc.dram_tensor("wi", [128, T * 512], BF16, kind="ExternalInput")
    d_bq = nc.dram_tensor("bq", [CD, HL], F32, kind="ExternalInput")
    d_bk = nc.dram_tensor("bk", [CD, HL], F32, kind="ExternalInput")
    d_bg = nc.dram_tensor("bg", [128, 4], F32, kind="ExternalInput")
    d_bi = nc.dram_tensor("bi", [128, 4], F32, kind="ExternalInput")
    d_out = nc.dram_tensor("out", [4, 128, S], F32, kind="ExternalOutput")

    with tile.TileContext(nc) as tc:
        with (
            tc.tile_pool(name="xin", bufs=1) as p_xin,
            tc.tile_pool(name="wts", bufs=1) as p_w,
            tc.tile_pool(name="big", bufs=1) as p_big,
            tc.tile_pool(name="att", bufs=1) as p_att,
            tc.tile_pool(name="tail", bufs=1) as p_tail,
            tc.tile_pool(name="psA", bufs=1, space="PSUM") as p_psA,
            tc.tile_pool(name="psS", bufs=1, space="PSUM") as p_psS,
            tc.tile_pool(name="psX", bufs=1, space="PSUM") as p_psX,
            tc.tile_pool(name="dram", bufs=1, space="DRAM") as p_dram,
        ):
            # --- persistent sbuf tiles (tags control slot reuse) ---
            t_xv = p_xin.tile([128, T, S], BF16, tag="vin", bufs=1)
            t_xqa = p_xin.tile([128, T, S], BF16, tag="qin", bufs=2)
            t_xqg = p_xin.tile([128, T, S], BF16, tag="qin", bufs=2)
            t_xka = p_xin.tile([128, T, S], BF16, tag="kin", bufs=2)
            t_xkg = p_xin.tile([128, T, S], BF16, tag="kin", bufs=2)

            t_wv = p_w.tile([128, T, 512], BF16, tag="w8", bufs=5)
            t_wqa = p_w.tile([128, T, 512], BF16, tag="w8", bufs=5)
            t_wqg = p_w.tile([128, T, 512], BF16, tag="w8", bufs=5)
            t_wka = p_w.tile([128, T, 512], BF16, tag="w8", bufs=5)
            t_wkg = p_w.tile([128, T, 512], BF16, tag="w8", bufs=5)
            t_wi = p_w.tile([128, T, 512], BF16, tag="w8", bufs=5)

            t_bq = p_w.tile([CD, HL], F32, tag="bias", bufs=4)
            t_bk = p_w.tile([CD, HL], F32, tag="bias", bufs=4)
            t_bg = p_w.tile([128, 4], F32, tag="bias", bufs=4)
            t_bi = p_w.tile([128, 4], F32, tag="bias", bufs=4)

            t_va = p_att.tile([128, T, HL, DK + 1], BF16, tag="va", bufs=1)
            # gathered x (all 8 x-dim blocks; the local half is written by
            # the normalize mult, then the exchange re-writes it (same
            # bytes) and fills the partner half)
            t_xtf = p_big.tile([128, T, S], BF16, tag="xtf", bufs=1)
            t_graw = p_w.tile([128, 4, S], BF16, tag="w8", bufs=5)

            # --- PE warmup: keep TensorE busy during the DMA lead-in so HAM
            # un-throttles before the real matmuls arrive ---
            t_wu = p_att.tile([128, 256], BF16, tag="wu", bufs=1)
            nc.vector.memset(t_wu[:], 0.0)
            for _ in range(24):
                pwu = p_psA.tile([128, 256], F32, tag="proj", bufs=2)
                nc.tensor.matmul(pwu[:], t_wu[:, 0:128], t_wu[:],
                                 start=True, stop=True)

            # --- load inputs / weights, in consumption order; 1MB chunks
            # so several DMA queues run in parallel ---
            def load(dt_, tl, n_t, chunk=4):
                r = dt_.ap().rearrange("p (t n) -> p t n", t=n_t)
                for tt in range(0, n_t, chunk):
                    nc.sync.dma_start(tl[:, tt:tt + chunk, :],
                                      r[:, tt:tt + chunk, :])

            def load2(da, ta, db, tb, n_t, chunk=2):
                ra = da.ap().rearrange("p (t n) -> p t n", t=n_t)
                rb = db.ap().rearrange("p (t n) -> p t n", t=n_t)
                for tt in range(0, n_t, chunk):
                    nc.sync.dma_start(ta[:, tt:tt + chunk, :],
                                      ra[:, tt:tt + chunk, :])
                    nc.sync.dma_start(tb[:, tt:tt + chunk, :],
                                      rb[:, tt:tt + chunk, :])

            load(d_wv, t_wv, T)
            load(d_xv, t_xv, T)
            load2(d_wqa, t_wqa, d_wqg, t_wqg, T)
            load2(d_xqa, t_xqa, d_xqg, t_xqg, T)
            load2(d_wka, t_wka, d_wkg, t_wkg, T)
            load2(d_xka, t_xka, d_xkg, t_xkg, T)
            t_wg = p_xin.tile([128, 2 * T, 512], BF16, tag="vin", bufs=1)
            load(d_wg, t_wg, 2 * T)
            load(d_wi, t_wi, T)
            nc.sync.dma_start(t_bq[:], d_bq.ap())
            nc.sync.dma_start(t_bk[:], d_bk.ap())
            nc.sync.dma_start(t_bg[:], d_bg.ap())
            nc.sync.dma_start(t_bi[:], d_bi.ap())

            # --- q/k projections (transposed, concat layout) ---
            # psum rows 0:64 <- a-head dims (weights col-tile 0), rows 64:128
            # <- g-head dims (col-tile 64); the two M=64 matmuls per step run
            # concurrently in distinct PE column groups.
            def proj_head(h, wa, wb, xa, xb, bias, name):
                nbufs = 8 if name == "qT" else 3
                dst = p_big.tile([128, S], BF16, tag=name, bufs=nbufs,
                                 name=f"{name}_{h}")
                for n in range(NQ):
                    ps = p_psA.tile([128, 512], F32, tag="proj", bufs=2)
                    for kt in range(T):
                        nc.tensor.matmul(
                            ps[0:64, :],
                            wa[:, kt, h * DK:(h + 1) * DK],
                            xa[:, kt, n * 512:(n + 1) * 512],
                            start=(kt == 0), stop=(kt == T - 1),
                            tile_position=(0, 0), skip_group_check=True,
                        )
                        nc.tensor.matmul(
                            ps[64:128, :],
                            wb[:, kt, h * DK:(h + 1) * DK],
                            xb[:, kt, n * 512:(n + 1) * 512],
                            start=(kt == 0), stop=(kt == T - 1),
                            tile_position=(0, 64), skip_group_check=True,
                        )
                    nc.vector.tensor_scalar_add(
                        dst[:, n * 512:(n + 1) * 512], ps[:],
                        bias[:, h:h + 1],
                    )
                return dst

            # --- va projection (natural [s, dk] layout, + ones column) ---
            nc.vector.memset(t_va[:, :, :, DK:DK + 1], 1.0)
            for st in range(T):
                ps = p_psA.tile([128, 512], F32, tag="proj", bufs=2)
                for kt in range(T):
                    nc.tensor.matmul(
                        ps[:],
                        t_xv[:, kt, st * 128:(st + 1) * 128],
                        t_wv[:, kt, :],
                        start=(kt == 0), stop=(kt == T - 1),
                    )
                nc.vector.tensor_copy(
                    t_va[:, st, :, 0:DK],
                    ps[:].rearrange("p (h d) -> p h d", h=HL),
                )

            # fillers pinned behind va bridge the xka/xkg DMA window
            for _ in range(12):
                pwu = p_psA.tile([128, 256], F32, tag="proj", bufs=2,
                                 name="pwu_fill")
                nc.tensor.matmul(pwu[0:DK + 1, :], t_va[:, 0, 0, :], t_wu[:],
                                 start=True, stop=True)

            # q for all heads first: depends only on the earliest DMAs, so
            # the PE has dense work while the k/v inputs stream in
            all_q = [proj_head(h, t_wqa, t_wqg, t_xqa, t_xqg, t_bq, "qT")
                     for h in range(HL)]

            # fillers pinned behind the q block bridge the xv DMA window
            for _ in range(8):
                pwu = p_psA.tile([128, 256], F32, tag="proj", bufs=2,
                                 name="pwu_fq")
                nc.tensor.matmul(pwu[:], all_q[7][:, 0:128], t_wu[:],
                                 start=True, stop=True)

            # --- attention (k-major scores, flash-style over k tiles) ---
            def attn_head(h, t_qh, t_kh):
                px = p_psX.tile([DK + 1, S], F32, tag="x", bufs=2,
                                name=f"px_{h}")
                for kt in range(T):
                    te = p_att.tile([128, S], BF16, tag="exp", bufs=3,
                                    name=f"te_{h}_{kt}")
                    for n in range(NQ):
                        pss = p_psS.tile([128, 512], F32, tag="s", bufs=2,
                                         name=f"pss_{h}_{kt}_{n}")
                        nc.tensor.matmul(
                            pss[:],
                            t_kh[:, kt * 128:(kt + 1) * 128],
                            t_qh[:, n * 512:(n + 1) * 512],
                            start=True, stop=True,
                        )
                        nc.scalar.activation(
                            te[:, n * 512:(n + 1) * 512], pss[:],
                            mybir.ActivationFunctionType.Exp, scale=SCALE,
                        )
                    for n in range(NQ):
                        nc.tensor.matmul(
                            px[:, n * 512:(n + 1) * 512],
                            t_va[:, kt, h, :],
                            te[:, n * 512:(n + 1) * 512],
                            start=(kt == 0), stop=(kt == T - 1),
                        )
                # normalize: row DK of px holds the softmax denominator.
                # The reciprocal runs on the (otherwise idle) DVE, keeping
                # the ACT table set pinned to exp all through attention.
                # (The custom DVE op misreads PSUM at non-zero bank offsets,
                # so the row is staged through SBUF first.)
                t_den = p_att.tile([1, S], F32, tag="den", bufs=1,
                                   name=f"den_{h}")
                nc.vector.tensor_copy(t_den[:], px[DK:DK + 1, :])
                t_rcp = p_att.tile([1, S], F32, tag="recip", bufs=1,
                                   name=f"recip_{h}")
                nc.vector.reciprocal_approx_fast(t_rcp[:], t_den[:])
                for n in range(NQ):
                    t_bc = p_att.tile([DK, 512], F32, tag="bc", bufs=1,
                                      name=f"bc_{h}_{n}")
                    nc.gpsimd.partition_broadcast(
                        t_bc[:], t_rcp[0:1, n * 512:(n + 1) * 512])
                    mult_inst = nc.vector.tensor_tensor(
                        t_xtf[(h % 2) * DK:(h % 2) * DK + DK, h // 2,
                              n * 512:(n + 1) * 512],
                        px[0:DK, n * 512:(n + 1) * 512], t_bc[:],
                        op=mybir.AluOpType.mult,
                    )
                return mult_inst

            # Ship each head's x block with its own pairwise AllGather as
            # soon as it completes: early heads hide entirely under the rest
            # of attention; the last head's exchange is covered by the gate
            # sigmoids and the info tail below.
            def ship_head(h):
                lo = (h % 2) * DK
                cc_in = p_dram.tile([1, DK, S], BF16, name=f"cci_{h}")
                cc_out = p_dram.tile([2, DK, S], BF16, name=f"cco_{h}")
                nc.sync.dma_start(cc_in[0], t_xtf[lo:lo + DK, h // 2, :])
                nc.gpsimd.collective_compute(
                    "AllGather", mybir.AluOpType.bypass,
                    replica_groups=REPLICA_GROUPS,
                    ins=[cc_in[:].opt()], outs=[cc_out[:].opt()],
                )
                nc.sync.dma_start(t_xtf[lo:lo + DK, h // 2, :], cc_out[0])
                nc.sync.dma_start(t_xtf[lo:lo + DK, 4 + h // 2, :], cc_out[1])

            head_done = []
            for h in range(HL):
                t_kh = proj_head(h, t_wka, t_wkg, t_xka, t_xkg, t_bk, "kT")
                head_done.append(attn_head(h, all_q[h], t_kh))
                ship_head(h)

            # --- gate matmuls (fill PE slack in the ACT-paced attention);
            # raw scores stashed, sigmoids batched at the tail so the ACT
            # pays a single table switch, inside the last-exchange window ---
            def gate_group(mt, n, anchor):
                ps = p_psA.tile([128, 512], F32, tag="proj", bufs=2,
                                name=f"gate_{mt}_{n}")
                for kt in range(2 * T):
                    xsrc = t_xqa if kt < T else t_xqg
                    mm = nc.tensor.matmul(
                        ps[:],
                        t_wg[:, kt, mt * 128:(mt + 1) * 128],
                        xsrc[:, kt % T, n * 512:(n + 1) * 512],
                        start=(kt == 0), stop=(kt == 2 * T - 1),
                    )
                    if kt == 0 and anchor is not None:
                        add_dep_helper(mm.ins, anchor.ins, sync=True,
                                       reason="gate after mid-attention")
                nc.vector.tensor_copy(
                    t_graw[:, mt, n * 512:(n + 1) * 512], ps[:])

            # scheduling-only anchors: gate fills the stalls of the LAST few
            # attention heads and the exchange window, never the early heads
            # (which set the pace of the serializing per-head AllGathers)
            gate_anchor = [2, 3, 4, 5, 5, 6, 6, 6]
            for i, (mt, n) in enumerate((m, nn) for m in range(4)
                                        for nn in range(NQ)):
                gate_group(mt, n, head_done[gate_anchor[i]])

            t_gate = p_xin.tile([128, 4, S], BF16, tag="kin", bufs=2)
            for mt in range(4):
                nc.scalar.activation(
                    t_gate[:, mt, :], t_graw[:, mt, :],
                    mybir.ActivationFunctionType.Sigmoid,
                    bias=t_bg[:, mt:mt + 1],
                )

            # --- info, split so only x-blocks 3/7 are contracted after the
            # last AllGather lands; blocks {0,4,1,5,2,6} pre-accumulate into
            # SBUF during late attention ---
            t_acc = p_tail.tile([128, 4, NQ, 512], BF16, tag="acc", bufs=1)
            for mt in range(4):
                for n in range(NQ):
                    ps = p_psA.tile([128, 512], F32, tag="proj", bufs=2,
                                    name=f"infoA_{mt}_{n}")
                    for i, kt in enumerate((0, 4, 1, 5, 2, 6)):
                        mm = nc.tensor.matmul(
                            ps[:],
                            t_wi[:, kt, mt * 128:(mt + 1) * 128],
                            t_xtf[:, kt, n * 512:(n + 1) * 512],
                            start=(i == 0), stop=(i == 5),
                        )
                        if i == 0:
                            add_dep_helper(mm.ins, head_done[5].ins, sync=True,
                                           reason="infoA after head 5")
                    nc.vector.tensor_copy(t_acc[:, mt, n, :], ps[:])
            for mt in range(4):
                for n in range(NQ):
                    ps = p_psA.tile([128, 512], F32, tag="proj", bufs=2,
                                    name=f"infoB_{mt}_{n}")
                    for i, kt in enumerate((3, 7)):
                        nc.tensor.matmul(
                            ps[:],
                            t_wi[:, kt, mt * 128:(mt + 1) * 128],
                            t_xtf[:, kt, n * 512:(n + 1) * 512],
                            start=(i == 0), stop=(i == 1),
                        )
                    t_sb = p_tail.tile([128, 512], F32, tag="osum", bufs=2)
                    nc.vector.scalar_tensor_tensor(
                        t_sb[:], ps[:], t_bi[:, mt:mt + 1],
                        t_acc[:, mt, n, :],
                        op0=mybir.AluOpType.add, op1=mybir.AluOpType.add,
                    )
                    t_ob = p_tail.tile([128, 512], F32, tag="outb", bufs=2)
                    nc.vector.tensor_tensor(
                        t_ob[:], t_sb[:],
                        t_gate[:, mt, n * 512:(n + 1) * 512],
                        op=mybir.AluOpType.mult,
                    )
                    nc.sync.dma_start(
                        d_out.ap()[mt, :, n * 512:(n + 1) * 512], t_ob[:])

    nc.compile()
    return nc


def make_in_maps(inputs):
    """Host-side sharding: transpose/slice/cast the full inputs per core."""
    f32 = np.float32
    g = {k: np.asarray(v) for k, v in inputs.items()}
    binfo_eff = (
        g["binfo"].astype(np.float64)
        + g["Winfo"].astype(np.float64) @ g["bva"].astype(np.float64)
    ).astype(f32)

    in_maps = []
    for c in range(8):
        b, hh = c // 2, c % 2
        hs = slice(hh * 512, (hh + 1) * 512)

        def pmajor(a):
            # [1024*k, n] -> partition-major [128, k*T*n]-style layout the
            # kernel DMAs as long contiguous per-partition lines
            rows, n = a.shape
            t = rows // 128
            return np.ascontiguousarray(
                a.reshape(t, 128, n).transpose(1, 0, 2).reshape(128, t * n))

        def xt(name):
            return pmajor(g[name][b].T.astype(NPBF16))

        def wt(name):
            return pmajor(g[name][hs].T.astype(NPBF16))

        def bqk(pa, pg):
            a = g[pa][hs].reshape(HL, DK).T.astype(f32)   # [64, 8]
            gg = g[pg][hs].reshape(HL, DK).T.astype(f32)
            return np.ascontiguousarray(np.vstack([a, gg]))  # [128, 8]

        m = {
            "xqa": xt("query_a"), "xqg": xt("query_g"),
            "xka": xt("key_a"), "xkg": xt("key_g"), "xv": xt("value_a"),
            "wqa": wt("Wqa"), "wqg": wt("Wqg"),
            "wka": wt("Wka"), "wkg": wt("Wkg"), "wv": wt("Wva"),
            "wg": wt("Wgate"), "wi": wt("Winfo"),
            "bq": bqk("bqa", "bqg"), "bk": bqk("bka", "bkg"),
            "bg": np.ascontiguousarray(
                g["bgate"][hs].reshape(4, 128).T.astype(f32)),
            "bi": np.ascontiguousarray(
                binfo_eff[hs].reshape(4, 128).T.astype(f32)),
        }
        in_maps.append(m)
    return in_maps


def assemble(results):
    out = np.empty((B, S, D), dtype=np.float32)
    for c in range(8):
        b, hh = c // 2, c % 2
        blk = results[c]["out"].reshape(512, S)   # [cols, seq]
        out[b, :, hh * 512:(hh + 1) * 512] = blk.T
    return out


_NC_CACHE = {}


def _get_nc():
    if "nc" not in _NC_CACHE:
        _NC_CACHE["nc"] = build_nc()
    return _NC_CACHE["nc"]


LAST_RESULTS = None


def kernel(**inputs) -> np.ndarray:
    global LAST_RESULTS
    nc = _get_nc()
    in_maps = make_in_maps(inputs)
    trace = os.environ.get("KERNEL_TRACE", "0") == "1"
    kwargs = {}
    if trace:
        kwargs["trace_cores"] = list(range(8))
    res = bass_utils.run_bass_kernel_spmd(
        nc, in_maps, core_ids=list(range(8)), trace=trace, **kwargs,
    )
    LAST_RESULTS = res
    return assemble(res.results)
